# revision 2
# baseline (speedup 1.0000x reference)
"""Bass/Tile TRN2 kernel for bilinear-score attention (score softmax + context).

reference:
    qW     = query @ W                      [B, Tq, Dk]
    weight = qW @ keys^T + mask[:, None, :] [B, Tq, Tk]
    score  = softmax(weight, axis=-1)
    ctx    = score @ values                 [B, Tq, Dv]
    returns (score, ctx)

Sharding: data-parallel over batch B=16 across 8 NeuronCores (2 batches/core).
Numerics: fp16 hi/lo 3-pass matmuls (hh + hl + lh) for both big contractions
(near-fp32 logits); phase 3 uses one float32r x float32r pass (score
reconstructed fp32 from transposed fp16, values truncated to ~12-bit mantissa
by the PE read path).
"""

import os
import sys

import numpy as np

os.environ.setdefault("JAX_COMPILATION_CACHE_DIR", "/tmp/jax_comp_cache")

for _p in ("/opt/trn_rl_repo",):
    if _p not in sys.path and os.path.isdir(_p):
        sys.path.insert(0, _p)

import concourse.bass as bass  # noqa: E402
import concourse.tile as tile  # noqa: E402
from concourse import bacc, mybir  # noqa: E402
from concourse.bass import ds, ts  # noqa: E402
from concourse.bass_utils import run_bass_kernel_spmd  # noqa: E402

P = 128
T = 1024
NT = T // P  # 8
NB = 2       # batches per core
NCORES = 8
F32 = mybir.dt.float32
F16 = mybir.dt.float16
F32R = mybir.dt.float32r
AX = mybir.AxisListType
AOP = mybir.AluOpType
AF = mybir.ActivationFunctionType


def _hi_lo_to_scratch(nc, stage, src_ap, hi_scr, lo_scr):
    """Load fp32 rows, split into fp16 hi/lo, store to DRAM scratch."""
    for t in range(NT):
        xf = stage.tile([P, T], F32, tag="ldf32")
        nc.sync.dma_start(xf[:], src_ap[ts(t, P), :])
        xh = stage.tile([P, T], F16, tag="hi16")
        nc.vector.tensor_copy(xh[:], xf[:])
        xl = stage.tile([P, T], F16, tag="lo16")
        nc.vector.tensor_tensor(xl[:], xf[:], xh[:], AOP.subtract)
        nc.sync.dma_start(hi_scr[ts(t, P), :], xh[:])
        nc.sync.dma_start(lo_scr[ts(t, P), :], xl[:])


def _phase1(nc, pools, b, w_hi_scr, w_lo_scr, qTh, qTl):
    """qWT[e, q] = W^T @ query^T as fp16 hi/lo, 3-pass per psum tile."""
    wtile, qwt_pool, psA = pools["wtile"], pools["qwt"], pools["psA"]
    qWTh = qwt_pool.tile([P, NT, T], F16, tag="qWTh")
    qWTl = qwt_pool.tile([P, NT, T], F16, tag="qWTl")
    for ec in range(4):  # 256-wide e chunks of W
        wch = wtile.tile([P, NT, 256], F16, tag="wch")
        wcl = wtile.tile([P, NT, 256], F16, tag="wcl")
        nc.sync.dma_start(
            wch[:], w_hi_scr[:, ds(ec * 256, 256)].rearrange("(o p) e -> p o e", p=P)
        )
        nc.sync.dma_start(
            wcl[:], w_lo_scr[:, ds(ec * 256, 256)].rearrange("(o p) e -> p o e", p=P)
        )
        for eh in range(2):
            et = ec * 2 + eh
            ps = psA.tile([P, T], F32, tag="psA")
            for dt_ in range(NT):
                lw_h = wch[:, dt_, ds(eh * P, P)]
                lw_l = wcl[:, dt_, ds(eh * P, P)]
                first = dt_ == 0
                last = dt_ == NT - 1
                for qc in range(2):
                    nc.tensor.matmul(
                        ps[:, ds(qc * 512, 512)], lw_h,
                        qTh[:, dt_, ds(qc * 512, 512)], start=first, stop=False,
                    )
                for qc in range(2):
                    nc.tensor.matmul(
                        ps[:, ds(qc * 512, 512)], lw_h,
                        qTl[:, dt_, ds(qc * 512, 512)], start=False, stop=False,
                    )
                for qc in range(2):
                    nc.tensor.matmul(
                        ps[:, ds(qc * 512, 512)], lw_l,
                        qTh[:, dt_, ds(qc * 512, 512)], start=False, stop=last,
                    )
            nc.scalar.copy(qWTh[:, et, :], ps[:])
            nc.vector.tensor_tensor(qWTl[:, et, :], ps[:], qWTh[:, et, :], AOP.subtract)
    return qWTh, qWTl


def _phase2_softmax(nc, pools, b, s_d, qWTh, qWTl, kTh, kTl, ones, mrep, s16_scr):
    """weight[q, k] = qW @ keys^T + mask; softmax rows; write score + fp16 copy."""
    soft, sc_pool, psB = pools["soft"], pools["sc"], pools["psB"]
    for qt_ in range(NT):
        ps2 = psB.tile([P, T], F32, tag="psB")
        for et in range(NT):
            for li, (lhs, rhs) in enumerate(((qWTh, kTh), (qWTh, kTl), (qWTl, kTh))):
                lw = lhs[:, et, ts(qt_, P)]
                for kc in range(2):
                    nc.tensor.matmul(
                        ps2[:, ds(kc * 512, 512)], lw, rhs[:, et, ds(kc * 512, 512)],
                        start=(et == 0 and li == 0), stop=False,
                    )
        for kc in range(2):
            nc.tensor.matmul(
                ps2[:, ds(kc * 512, 512)], ones[:], mrep[:, ds(kc * 512, 512)],
                start=False, stop=True,
            )
        negmax = soft.tile([P, 1], F32, tag="negmax")
        nc.vector.tensor_reduce(negmax[:], ps2[:], axis=AX.X, op=AOP.max, negate=True)
        expt = soft.tile([P, T], F32, tag="expt")
        sumexp = soft.tile([P, 1], F32, tag="sumexp")
        nc.scalar.activation(
            expt[:], ps2[:], AF.Exp, bias=negmax[:], scale=1.0, accum_out=sumexp[:]
        )
        recip = soft.tile([P, 1], F32, tag="recip")
        nc.vector.reciprocal(recip[:], sumexp[:])
        nc.vector.tensor_scalar_mul(expt[:], expt[:], recip[:])
        nc.sync.dma_start(s_d[b, ts(qt_, P), :], expt[:])
        s16 = sc_pool.tile([P, T], F16, tag="s16t")
        nc.scalar.copy(s16[:], expt[:])
        nc.sync.dma_start(s16_scr[ts(qt_, P), :], s16[:])


def _phase3(nc, pools, b, c_d, s16_scr, vals):
    """ctx[q, v] = score @ values via one f32r x f32r pass."""
    st_pool, cx_pool, psA = pools["st"], pools["cx"], pools["psA"]
    for qt_ in range(NT):
        sT16 = st_pool.tile([P, NT, P], F16, tag="sT16")
        for kt_ in range(NT):
            nc.sync.dma_start_transpose(
                sT16[:, kt_, :], s16_scr[ts(qt_, P), ts(kt_, P)]
            )
        sTr = st_pool.tile([P, NT, P], F32R, tag="sTr")
        nc.vector.tensor_copy(sTr[:], sT16[:])
        ps3 = psA.tile([P, T], F32, tag="psA")
        for kt_ in range(NT):
            lw = sTr[:, kt_, :]
            for vc in range(2):
                nc.tensor.matmul(
                    ps3[:, ds(vc * 512, 512)], lw, vals[:, kt_, ds(vc * 512, 512)],
                    start=(kt_ == 0), stop=(kt_ == NT - 1),
                )
        cx = cx_pool.tile([P, T], F32, tag="cx")
        nc.scalar.copy(cx[:], ps3[:])
        nc.sync.dma_start(c_d[b, ts(qt_, P), :], cx[:])


def _batch(nc, pools, b, tensors, w_hi_scr, w_lo_scr, ones):
    q_d, k_d, v_d, m_d, s_d, c_d = tensors
    stage, small, dram = pools["stage"], pools["small"], pools["dram"]
    qt_pool, kt_pool, val_pool = pools["qt"], pools["kt"], pools["val"]

    # mask -> fp16 broadcast to all partitions
    mf = stage.tile([P, T], F32, tag="ldf32")
    nc.sync.dma_start(mf[:1, :], m_d[b : b + 1, :])
    m16 = small.tile([1, T], F16, tag="mask16")
    nc.vector.tensor_copy(m16[:], mf[:1, :])
    mrep = small.tile([P, T], F16, tag="mrep")
    nc.gpsimd.partition_broadcast(mrep[:], m16[:])

    # query/keys hi/lo staging
    q_hi_scr = dram.tile([T, T], F16, tag="qhi")
    q_lo_scr = dram.tile([T, T], F16, tag="qlo")
    k_hi_scr = dram.tile([T, T], F16, tag="khi")
    k_lo_scr = dram.tile([T, T], F16, tag="klo")
    _hi_lo_to_scratch(nc, stage, q_d[b], q_hi_scr, q_lo_scr)
    _hi_lo_to_scratch(nc, stage, k_d[b], k_hi_scr, k_lo_scr)

    # transposed fp16 loads
    qTh = qt_pool.tile([P, NT, T], F16, tag="qTh")
    qTl = qt_pool.tile([P, NT, T], F16, tag="qTl")
    kTh = kt_pool.tile([P, NT, T], F16, tag="kTh")
    kTl = kt_pool.tile([P, NT, T], F16, tag="kTl")
    for dt_ in range(NT):
        nc.sync.dma_start_transpose(qTh[:, dt_, :], q_hi_scr[:, ts(dt_, P)])
        nc.sync.dma_start_transpose(qTl[:, dt_, :], q_lo_scr[:, ts(dt_, P)])
        nc.sync.dma_start_transpose(kTh[:, dt_, :], k_hi_scr[:, ts(dt_, P)])
        nc.sync.dma_start_transpose(kTl[:, dt_, :], k_lo_scr[:, ts(dt_, P)])

    # values as float32r
    vals = val_pool.tile([P, NT, T], F32R, tag="vals")
    for kt_ in range(NT):
        nc.sync.dma_start(vals[:, kt_, :], v_d[b, ts(kt_, P), :])

    qWTh, qWTl = _phase1(nc, pools, b, w_hi_scr, w_lo_scr, qTh, qTl)
    s16_scr = dram.tile([T, T], F16, tag="s16")
    _phase2_softmax(nc, pools, b, s_d, qWTh, qWTl, kTh, kTl, ones, mrep, s16_scr)
    _phase3(nc, pools, b, c_d, s16_scr, vals)


def build_nc():
    nc = bacc.Bacc("TRN2", target_bir_lowering=False, debug=False, num_devices=NCORES)
    q_d = nc.dram_tensor("query", [NB, T, T], F32, kind="ExternalInput")
    k_d = nc.dram_tensor("keys", [NB, T, T], F32, kind="ExternalInput")
    v_d = nc.dram_tensor("values", [NB, T, T], F32R, kind="ExternalInput")
    w_d = nc.dram_tensor("W", [T, T], F32, kind="ExternalInput")
    m_d = nc.dram_tensor("mask", [NB, T], F32, kind="ExternalInput")
    s_d = nc.dram_tensor("score", [NB, T, T], F32, kind="ExternalOutput")
    c_d = nc.dram_tensor("ctx", [NB, T, T], F32, kind="ExternalOutput")

    with tile.TileContext(nc) as tc:
        with (
            tc.tile_pool(name="stage", bufs=2) as stage,
            tc.tile_pool(name="wtile", bufs=2) as wtile,
            tc.tile_pool(name="qt", bufs=1) as qt_pool,
            tc.tile_pool(name="qwt", bufs=1) as qwt_pool,
            tc.tile_pool(name="kt", bufs=1) as kt_pool,
            tc.tile_pool(name="val", bufs=1) as val_pool,
            tc.tile_pool(name="soft", bufs=2) as soft,
            tc.tile_pool(name="sc", bufs=2) as sc_pool,
            tc.tile_pool(name="st", bufs=2) as st_pool,
            tc.tile_pool(name="cx", bufs=2) as cx_pool,
            tc.tile_pool(name="small", bufs=2) as small,
            tc.tile_pool(name="ones", bufs=1) as ones_pool,
        ):
            with (
                tc.tile_pool(name="psA", bufs=2, space="PSUM") as psA,
                tc.tile_pool(name="psB", bufs=2, space="PSUM") as psB,
                tc.tile_pool(name="dram", bufs=2, space="DRAM") as dram,
                tc.tile_pool(name="dramw", bufs=1, space="DRAM") as dramw,
            ):
                pools = {
                    "stage": stage, "wtile": wtile, "qt": qt_pool, "qwt": qwt_pool,
                    "kt": kt_pool, "val": val_pool, "soft": soft, "sc": sc_pool,
                    "st": st_pool, "cx": cx_pool, "small": small,
                    "psA": psA, "psB": psB, "dram": dram,
                }
                ones = ones_pool.tile([P, P], F16)
                nc.vector.memset(ones[:], 1.0 / P)

                # W -> hi/lo fp16 DRAM scratch (once per core)
                w_hi_scr = dramw.tile([T, T], F16)
                w_lo_scr = dramw.tile([T, T], F16)
                _hi_lo_to_scratch(nc, stage, w_d, w_hi_scr, w_lo_scr)

                tensors = (q_d, k_d, v_d, m_d, s_d, c_d)
                for b in range(NB):
                    _batch(nc, pools, b, tensors, w_hi_scr, w_lo_scr, ones)

    nc.compile()
    return nc


_nc = None


def _get_nc():
    global _nc
    if _nc is None:
        _nc = build_nc()
    return _nc


def make_in_maps(query, keys, values, W, mask):
    query = np.ascontiguousarray(np.asarray(query, dtype=np.float32))
    keys = np.ascontiguousarray(np.asarray(keys, dtype=np.float32))
    values = np.ascontiguousarray(np.asarray(values, dtype=np.float32))
    W = np.ascontiguousarray(np.asarray(W, dtype=np.float32))
    mask = np.ascontiguousarray(np.asarray(mask, dtype=np.float32))
    in_maps = []
    for c in range(NCORES):
        sl = slice(c * NB, (c + 1) * NB)
        in_maps.append(
            {
                "query": query[sl],
                "keys": keys[sl],
                "values": values[sl],
                "W": W,
                "mask": mask[sl],
            }
        )
    return in_maps


def kernel(query, keys, values, W, mask):
    nc = _get_nc()
    in_maps = make_in_maps(query, keys, values, W, mask)
    res = run_bass_kernel_spmd(nc, in_maps, core_ids=list(range(NCORES)))
    score = np.concatenate([res.results[c]["score"] for c in range(NCORES)], axis=0)
    ctx = np.concatenate([res.results[c]["ctx"] for c in range(NCORES)], axis=0)
    return score, ctx


# revision 3
# speedup vs baseline: 42.1913x; 42.1913x over previous
"""Bass/Tile TRN2 kernel for bilinear-score attention (score softmax + context).

reference:
    qW     = query @ W                      [B, Tq, Dk]
    weight = qW @ keys^T + mask[:, None, :] [B, Tq, Tk]
    score  = softmax(weight, axis=-1)
    ctx    = score @ values                 [B, Tq, Dv]
    returns (score, ctx)

Sharding: data-parallel over batch B=16 across 8 NeuronCores (2 batches/core).
Numerics: fp16 hi/lo 3-pass matmuls (hh + hl + lh) for both big contractions
(near-fp32 logits); phase 3 uses one float32r x float32r pass (score
reconstructed fp32 from transposed fp16, values truncated to ~12-bit mantissa
by the PE read path).
"""

import os
import sys

import numpy as np

os.environ.setdefault("JAX_COMPILATION_CACHE_DIR", "/tmp/jax_comp_cache")

for _p in ("/opt/trn_rl_repo",):
    if _p not in sys.path and os.path.isdir(_p):
        sys.path.insert(0, _p)

import concourse.bass as bass  # noqa: E402
import concourse.tile as tile  # noqa: E402
from concourse import bacc, mybir  # noqa: E402
from concourse.bass import ds, ts  # noqa: E402
from concourse.bass_utils import run_bass_kernel_spmd  # noqa: E402

P = 128
T = 1024
NT = T // P  # 8
NB = 2       # batches per core
NCORES = 8
F32 = mybir.dt.float32
F16 = mybir.dt.float16
F32R = mybir.dt.float32r
AX = mybir.AxisListType
AOP = mybir.AluOpType
AF = mybir.ActivationFunctionType


def _hi_lo_to_scratch(nc, stage, src_ap, hi_scr, lo_scr):
    """Load fp32 rows, split into fp16 hi/lo, store to DRAM scratch."""
    for t in range(NT):
        xf = stage.tile([P, T], F32, tag="ldf32")
        nc.sync.dma_start(xf[:], src_ap[ts(t, P), :])
        xh = stage.tile([P, T], F16, tag="hi16")
        nc.vector.tensor_copy(xh[:], xf[:])
        xl = stage.tile([P, T], F16, tag="lo16")
        nc.vector.tensor_tensor(xl[:], xf[:], xh[:], AOP.subtract)
        nc.sync.dma_start(hi_scr[ts(t, P), :], xh[:])
        nc.sync.dma_start(lo_scr[ts(t, P), :], xl[:])


def _phase1(nc, pools, b, w_hi_scr, w_lo_scr, qTh, qTl):
    """qWT[e, q] = W^T @ query^T as fp16 hi/lo, 3-pass per psum tile."""
    wtile, qwt_pool, psA = pools["wtile"], pools["qwt"], pools["psA"]
    qWTh = qwt_pool.tile([P, NT, T], F16, tag="qWTh")
    qWTl = qwt_pool.tile([P, NT, T], F16, tag="qWTl")
    for ec in range(4):  # 256-wide e chunks of W
        wch = wtile.tile([P, NT, 256], F16, tag="wch")
        wcl = wtile.tile([P, NT, 256], F16, tag="wcl")
        nc.sync.dma_start(
            wch[:], w_hi_scr[:, ds(ec * 256, 256)].rearrange("(o p) e -> p o e", p=P)
        )
        nc.sync.dma_start(
            wcl[:], w_lo_scr[:, ds(ec * 256, 256)].rearrange("(o p) e -> p o e", p=P)
        )
        for eh in range(2):
            et = ec * 2 + eh
            ps = psA.tile([P, T], F32, tag="psA")
            for dt_ in range(NT):
                lw_h = wch[:, dt_, ds(eh * P, P)]
                lw_l = wcl[:, dt_, ds(eh * P, P)]
                first = dt_ == 0
                last = dt_ == NT - 1
                for qc in range(2):
                    nc.tensor.matmul(
                        ps[:, ds(qc * 512, 512)], lw_h,
                        qTh[:, dt_, ds(qc * 512, 512)], start=first, stop=False,
                    )
                for qc in range(2):
                    nc.tensor.matmul(
                        ps[:, ds(qc * 512, 512)], lw_h,
                        qTl[:, dt_, ds(qc * 512, 512)], start=False, stop=False,
                    )
                for qc in range(2):
                    nc.tensor.matmul(
                        ps[:, ds(qc * 512, 512)], lw_l,
                        qTh[:, dt_, ds(qc * 512, 512)], start=False, stop=last,
                    )
            nc.scalar.copy(qWTh[:, et, :], ps[:])
            nc.vector.tensor_tensor(qWTl[:, et, :], ps[:], qWTh[:, et, :], AOP.subtract)
    return qWTh, qWTl


def _phase2_softmax(nc, pools, b, s_d, qWTh, qWTl, kTh, kTl, ones, mrep, s16_scr):
    """weight[q, k] = qW @ keys^T + mask; softmax rows; write score + fp16 copy."""
    soft, sc_pool, psB = pools["soft"], pools["sc"], pools["psB"]
    for qt_ in range(NT):
        ps2 = psB.tile([P, T], F32, tag="psB")
        for et in range(NT):
            for li, (lhs, rhs) in enumerate(((qWTh, kTh), (qWTh, kTl), (qWTl, kTh))):
                lw = lhs[:, et, ts(qt_, P)]
                for kc in range(2):
                    nc.tensor.matmul(
                        ps2[:, ds(kc * 512, 512)], lw, rhs[:, et, ds(kc * 512, 512)],
                        start=(et == 0 and li == 0), stop=False,
                    )
        for kc in range(2):
            nc.tensor.matmul(
                ps2[:, ds(kc * 512, 512)], ones[:], mrep[:, ds(kc * 512, 512)],
                start=False, stop=True,
            )
        negmax = soft.tile([P, 1], F32, tag="negmax")
        nc.vector.tensor_reduce(negmax[:], ps2[:], axis=AX.X, op=AOP.max, negate=True)
        expt = soft.tile([P, T], F32, tag="expt")
        sumexp = soft.tile([P, 1], F32, tag="sumexp")
        nc.scalar.activation(
            expt[:], ps2[:], AF.Exp, bias=negmax[:], scale=1.0, accum_out=sumexp[:]
        )
        recip = soft.tile([P, 1], F32, tag="recip")
        nc.vector.reciprocal(recip[:], sumexp[:])
        nc.vector.tensor_scalar_mul(expt[:], expt[:], recip[:])
        nc.sync.dma_start(s_d[b, ts(qt_, P), :], expt[:])
        s16 = sc_pool.tile([P, T], F16, tag="s16t")
        nc.scalar.copy(s16[:], expt[:])
        nc.sync.dma_start(s16_scr[ts(qt_, P), :], s16[:])


def _phase3(nc, pools, b, c_d, s16_scr, vals):
    """ctx[q, v] = score @ values via one f32r x f32r pass."""
    st_pool, cx_pool, psA = pools["st"], pools["cx"], pools["psA"]
    for qt_ in range(NT):
        sT16 = st_pool.tile([P, NT, P], F16, tag="sT16")
        for kt_ in range(NT):
            nc.sync.dma_start_transpose(
                sT16[:, kt_, :], s16_scr[ts(qt_, P), ts(kt_, P)]
            )
        sTr = st_pool.tile([P, NT, P], F32R, tag="sTr")
        nc.vector.tensor_copy(sTr[:], sT16[:])
        ps3 = psA.tile([P, T], F32, tag="psA")
        for kt_ in range(NT):
            lw = sTr[:, kt_, :]
            for vc in range(2):
                nc.tensor.matmul(
                    ps3[:, ds(vc * 512, 512)], lw, vals[:, kt_, ds(vc * 512, 512)],
                    start=(kt_ == 0), stop=(kt_ == NT - 1),
                )
        cx = cx_pool.tile([P, T], F32, tag="cx")
        nc.scalar.copy(cx[:], ps3[:])
        nc.sync.dma_start(c_d[b, ts(qt_, P), :], cx[:])


def _batch(nc, pools, b, tensors, w_hi_scr, w_lo_scr, ones):
    q_d, k_d, v_d, m_d, s_d, c_d = tensors
    stage, small, dram = pools["stage"], pools["small"], pools["dram"]
    qt_pool, kt_pool, val_pool = pools["qt"], pools["kt"], pools["val"]

    # mask -> fp16 broadcast to all partitions
    mf = stage.tile([P, T], F32, tag="ldf32")
    nc.sync.dma_start(mf[:1, :], m_d[b : b + 1, :])
    m16 = small.tile([1, T], F16, tag="mask16")
    nc.vector.tensor_copy(m16[:], mf[:1, :])
    mrep = small.tile([P, T], F16, tag="mrep")
    nc.gpsimd.partition_broadcast(mrep[:], m16[:])

    # query/keys hi/lo staging
    q_hi_scr = dram.tile([T, T], F16, tag="qhi")
    q_lo_scr = dram.tile([T, T], F16, tag="qlo")
    k_hi_scr = dram.tile([T, T], F16, tag="khi")
    k_lo_scr = dram.tile([T, T], F16, tag="klo")
    _hi_lo_to_scratch(nc, stage, q_d[b], q_hi_scr, q_lo_scr)
    _hi_lo_to_scratch(nc, stage, k_d[b], k_hi_scr, k_lo_scr)

    # transposed fp16 loads
    qTh = qt_pool.tile([P, NT, T], F16, tag="qTh")
    qTl = qt_pool.tile([P, NT, T], F16, tag="qTl")
    kTh = kt_pool.tile([P, NT, T], F16, tag="kTh")
    kTl = kt_pool.tile([P, NT, T], F16, tag="kTl")
    for dt_ in range(NT):
        nc.sync.dma_start_transpose(qTh[:, dt_, :], q_hi_scr[:, ts(dt_, P)])
        nc.sync.dma_start_transpose(qTl[:, dt_, :], q_lo_scr[:, ts(dt_, P)])
        nc.sync.dma_start_transpose(kTh[:, dt_, :], k_hi_scr[:, ts(dt_, P)])
        nc.sync.dma_start_transpose(kTl[:, dt_, :], k_lo_scr[:, ts(dt_, P)])

    # values as float32r
    vals = val_pool.tile([P, NT, T], F32R, tag="vals")
    for kt_ in range(NT):
        nc.sync.dma_start(vals[:, kt_, :], v_d[b, ts(kt_, P), :])

    qWTh, qWTl = _phase1(nc, pools, b, w_hi_scr, w_lo_scr, qTh, qTl)
    s16_scr = dram.tile([T, T], F16, tag="s16")
    _phase2_softmax(nc, pools, b, s_d, qWTh, qWTl, kTh, kTl, ones, mrep, s16_scr)
    _phase3(nc, pools, b, c_d, s16_scr, vals)


def build_nc(reps=1):
    nc = bacc.Bacc("TRN2", target_bir_lowering=False, debug=False, num_devices=NCORES)
    q_d = nc.dram_tensor("query", [NB, T, T], F32, kind="ExternalInput")
    k_d = nc.dram_tensor("keys", [NB, T, T], F32, kind="ExternalInput")
    v_d = nc.dram_tensor("values", [NB, T, T], F32R, kind="ExternalInput")
    w_d = nc.dram_tensor("W", [T, T], F32, kind="ExternalInput")
    m_d = nc.dram_tensor("mask", [NB, T], F32, kind="ExternalInput")
    s_d = nc.dram_tensor("score", [NB, T, T], F32, kind="ExternalOutput")
    c_d = nc.dram_tensor("ctx", [NB, T, T], F32, kind="ExternalOutput")

    with tile.TileContext(nc) as tc:
        with (
            tc.tile_pool(name="stage", bufs=2) as stage,
            tc.tile_pool(name="wtile", bufs=2) as wtile,
            tc.tile_pool(name="qt", bufs=1) as qt_pool,
            tc.tile_pool(name="qwt", bufs=1) as qwt_pool,
            tc.tile_pool(name="kt", bufs=1) as kt_pool,
            tc.tile_pool(name="val", bufs=1) as val_pool,
            tc.tile_pool(name="soft", bufs=2) as soft,
            tc.tile_pool(name="sc", bufs=2) as sc_pool,
            tc.tile_pool(name="st", bufs=2) as st_pool,
            tc.tile_pool(name="cx", bufs=2) as cx_pool,
            tc.tile_pool(name="small", bufs=2) as small,
            tc.tile_pool(name="ones", bufs=1) as ones_pool,
        ):
            with (
                tc.tile_pool(name="psA", bufs=2, space="PSUM") as psA,
                tc.tile_pool(name="psB", bufs=2, space="PSUM") as psB,
                tc.tile_pool(name="dram", bufs=2, space="DRAM") as dram,
                tc.tile_pool(name="dramw", bufs=1, space="DRAM") as dramw,
            ):
                pools = {
                    "stage": stage, "wtile": wtile, "qt": qt_pool, "qwt": qwt_pool,
                    "kt": kt_pool, "val": val_pool, "soft": soft, "sc": sc_pool,
                    "st": st_pool, "cx": cx_pool, "small": small,
                    "psA": psA, "psB": psB, "dram": dram,
                }
                ones = ones_pool.tile([P, P], F16)
                nc.vector.memset(ones[:], 1.0 / P)

                # W -> hi/lo fp16 DRAM scratch (once per core)
                w_hi_scr = dramw.tile([T, T], F16)
                w_lo_scr = dramw.tile([T, T], F16)
                _hi_lo_to_scratch(nc, stage, w_d, w_hi_scr, w_lo_scr)

                tensors = (q_d, k_d, v_d, m_d, s_d, c_d)
                for _rep in range(reps):
                    for b in range(NB):
                        _batch(nc, pools, b, tensors, w_hi_scr, w_lo_scr, ones)

    nc.compile()
    return nc


_nc = None


def _get_nc():
    global _nc
    if _nc is None:
        _nc = build_nc()
    return _nc


def make_in_maps(query, keys, values, W, mask):
    query = np.ascontiguousarray(np.asarray(query, dtype=np.float32))
    keys = np.ascontiguousarray(np.asarray(keys, dtype=np.float32))
    values = np.ascontiguousarray(np.asarray(values, dtype=np.float32))
    W = np.ascontiguousarray(np.asarray(W, dtype=np.float32))
    mask = np.ascontiguousarray(np.asarray(mask, dtype=np.float32))
    in_maps = []
    for c in range(NCORES):
        sl = slice(c * NB, (c + 1) * NB)
        in_maps.append(
            {
                "query": query[sl],
                "keys": keys[sl],
                "values": values[sl],
                "W": W,
                "mask": mask[sl],
            }
        )
    return in_maps


def kernel(query, keys, values, W, mask):
    nc = _get_nc()
    in_maps = make_in_maps(query, keys, values, W, mask)
    res = run_bass_kernel_spmd(nc, in_maps, core_ids=list(range(NCORES)))
    score = np.concatenate([res.results[c]["score"] for c in range(NCORES)], axis=0)
    ctx = np.concatenate([res.results[c]["ctx"] for c in range(NCORES)], axis=0)
    return score, ctx
